# revision 40
# baseline (speedup 1.0000x reference)
"""Trainium2 Bass kernel for the CudaFastWeightPerformerLayer problem.

Algorithm: FAVOR+ features + delta-rule fast-weight recurrence, computed with
the chunked WY/UT-transform parallel form (chunk C=128, Neumann-2 solve of the
unit-triangular system). Sharding: core c handles batch b=c%2 and the 4 heads
[4*(c//2), 4*(c//2)+4).

Single fused dispatch, sized for the axon tunnel (wall time here is wire
bytes + a flat ~80ms per dispatch and per output tensor, while device-side
compute/collectives are ~free). Core c uploads h rows [(c//2)*512, +512) of
its batch as int4 (plane-packed nibbles: low nibble = cols 0-511, high = cols
512-1023) + one inline f32 absmax scale per row: 516B/row, ~2MB total. The
device unpacks to bf16, a grouped AllGather over [[0,2,4,6],[1,3,5,7]] of the
on-chip-transposed copy rebuilds the d-major sequence for the matmuls, the
chunked scan runs, and partial attn_out = outs_c @ W_o[head rows] is
ReduceScattered over each batch group. The scattered attn rows (NOT y: attn
has ~50x smaller magnitude, and keeping the residual in exact host f32
removes the input-quant error from the residual path) are int4-quantized
per row with inline f32 scales into a single merged output (~2MB down).
The host unpacks, adds the exact residual h, and applies layernorm in
numba. Weights/masks/zero-output buffers are device-cached after the first
call; dispatch goes through an AOT-compiled executable, and a retry-once
wrapper rebuilds device state after transient axon transport failures.

Self-contained: all shapes hardcoded; inputs are the full unsharded tensors.
"""
import numpy as np
import ml_dtypes

try:
    import numba

    # hs_all: (8*512, 516) uint8; hs_f32: aliased f32 view (8*512, 129).
    # Bytes 0-511 of each row hold int4 plane-packed values (low nibble =
    # cols 0-511, high nibble = cols 512-1023), bytes 512-515 the f32 scale
    # (disjoint byte ranges, so writing through both views is safe).
    @numba.njit(parallel=True, cache=True, fastmath=True)
    def _quant_nb(h4, hs_all, hs_f32, insum):
        for c in numba.prange(8):
            j = c // 2
            b = c % 2
            for r in range(512):
                row = h4[j, r, b]
                m = np.float32(0.0)
                for d in range(1024):
                    a = abs(row[d])
                    if a > m:
                        m = a
                sc = m * np.float32(1.0 / 7.0)
                if sc < np.float32(1e-30):
                    sc = np.float32(1e-30)
                hs_f32[c * 512 + r, 128] = sc
                rsc = np.float32(1.0) / sc
                orow = hs_all[c * 512 + r]
                tot = np.int32(0)
                for d in range(512):
                    q0 = np.int32(np.floor(row[d] * rsc + np.float32(7.5)))
                    q1 = np.int32(np.floor(row[d + 512] * rsc + np.float32(7.5)))
                    v = q0 + 16 * q1
                    tot += v
                    orow[d] = np.uint8(v)
                insum[c * 512 + r] = np.float32(tot)

    # yq: (8*512, 404) uint8 int3-packed attn rows (see the device-side
    # layout comment) + inline f32 absmax scale; host adds the exact
    # residual h and applies layernorm.
    @numba.njit(parallel=True, cache=True, fastmath=True)
    def _post_nb(yq, yq_f32, h4, gamma, beta, out):
        for c in numba.prange(8):
            j = c // 2
            b = c % 2
            for r in range(512):
                s = yq_f32[c * 512 + r, 96] * np.float32(1.0 / 3.0)
                q = yq[c * 512 + r]
                hrow = h4[j, r, b]
                o = out[j * 512 + r, b]
                ssum = np.float32(0.0)
                ssq = np.float32(0.0)
                for d in range(1024):
                    k = d >> 7
                    v2 = (np.int32(q[256 + (d & 127)]) >> k) & 1
                    rr = (np.int32(q[d & 255]) >> ((d >> 8) << 1)) & 3
                    x = hrow[d] + np.float32(4 * v2 + rr - 3) * s
                    o[d] = x
                    ssum += x
                    ssq += x * x
                mu = ssum * np.float32(1.0 / 1024.0)
                var = ssq * np.float32(1.0 / 1024.0) - mu * mu
                rstd = np.float32(1.0) / np.sqrt(var + np.float32(1e-5))
                for d in range(1024):
                    o[d] = (o[d] - mu) * rstd * gamma[d] + beta[d]
except Exception:  # numba unavailable: numpy fallback paths are used below
    _quant_nb = None
    _post_nb = None

SLEN, BSZ, D_MODEL, N_HEAD, D_HEAD, PROJ_DIM = 2048, 2, 1024, 16, 64, 256
LN_EPS = 1e-5
PRIME_EPS = 1e-4
P2M = 2 * PROJ_DIM          # 512 feature dim
C = 128                      # chunk length
NCHUNK = SLEN // C           # 16
HPC = 4                      # heads per core
N_CORES = 8
NEUMANN = 2
ROWS = SLEN // 4             # 512 seq rows per core (shard in + y out)

_cache = {}


def _build_fused():
    import concourse.bacc as bacc
    import concourse.mybir as mybir
    import concourse.tile as tile

    dt = mybir.dt
    AF = mybir.ActivationFunctionType
    nc = bacc.Bacc("TRN2", target_bir_lowering=False, debug=False)

    # merged input: 512 int4 plane-packed payload bytes + inline f32 scale
    hs = nc.dram_tensor("hs", (ROWS, D_MODEL // 2 + 4), dt.uint8, kind="ExternalInput").ap()
    hsf = hs.bitcast(dt.float32)  # (ROWS, 129); col 128 is the scale
    Wq = nc.dram_tensor("Wq", (D_MODEL, 256), dt.bfloat16, kind="ExternalInput").ap()
    Wk = nc.dram_tensor("Wk", (D_MODEL, 256), dt.bfloat16, kind="ExternalInput").ap()
    Wvb = nc.dram_tensor("Wvb", (D_MODEL, 260), dt.bfloat16, kind="ExternalInput").ap()
    pmA = nc.dram_tensor("pmA", (128, P2M), dt.bfloat16, kind="ExternalInput").ap()
    maskS = nc.dram_tensor("maskS", (128, 512), dt.float32, kind="ExternalInput").ap()
    maskI = nc.dram_tensor("maskI", (128, 512), dt.float32, kind="ExternalInput").ap()
    WoB = nc.dram_tensor("WoB", (256, D_MODEL), dt.bfloat16, kind="ExternalInput").ap()
    # merged output: 384 int3-packed attn bytes + 5 f32 metadata cols:
    # 96 = attn scale, 97 = attn scale duplicate, 98 = input payload
    # checksum echo, 99 = input scale echo, 100 = output payload checksum.
    # int3 packing: attn col d is quantized to q in [0,6], split q = 4*v2 + r
    # (v2 in {0,1}, r in [0,3]); byte j<256 packs r of cols {j+256m} at bit
    # pair 2m, byte 256+j (j<128) packs v2 of cols {j+128k} at bit k. All
    # on-device extractions are small-range exact f32 arithmetic.
    # The metadata lets the host detect silent transport corruption (observed
    # ~once per ~20 calls on the axon tunnel) and retry. A single output
    # tensor matters: each extra ExternalOutput adds ~80ms of fixed
    # per-dispatch overhead on the axon transport.
    yq = nc.dram_tensor("yq", (ROWS, 404), dt.uint8, kind="ExternalOutput").ap()
    yqf = yq.bitcast(dt.float32)  # (ROWS, 101)

    GRPS = [[0, 2, 4, 6], [1, 3, 5, 7]]
    cxn = float(D_HEAD ** -0.25)
    with tile.TileContext(nc) as tc:
        with (
            tc.tile_pool(name="dram", bufs=1, space="DRAM") as dram,
            tc.tile_pool(name="const", bufs=1) as cpool,
            tc.tile_pool(name="feat", bufs=1) as fpool,
            tc.tile_pool(name="kq", bufs=8) as kqpool,
            tc.tile_pool(name="small", bufs=3) as spool,
            tc.tile_pool(name="outp", bufs=3) as opool,
            tc.tile_pool(name="work", bufs=2) as wpool,
            tc.tile_pool(name="ln", bufs=1) as lnpool,
            tc.tile_pool(name="ps_big", bufs=1, space="PSUM") as psb,
            tc.tile_pool(name="ps_prj", bufs=2, space="PSUM") as psprj,
            tc.tile_pool(name="ps_v", bufs=1, space="PSUM") as psv,
        ):
            # ---- DRAM bounce buffers for collectives ----
            hTs_d = dram.tile([D_MODEL, ROWS], dt.bfloat16)           # transposed shard
            hTg_d = dram.tile([4 * D_MODEL, ROWS], dt.bfloat16)       # gathered hT
            P_d = dram.tile([SLEN, D_MODEL], dt.float32)              # partial attn
            R_d = dram.tile([ROWS, D_MODEL], dt.float32)              # reduce-scattered

            # ---- unpack own int4 shard to bf16; transpose it ----
            # byte = q0 | q1<<4 (planes: cols 0-511 / 512-1023); h = (q-7)*sc
            for ss in range(4):
                qt_in = wpool.tile([128, 512], dt.uint8, tag="qt_in")
                nc.sync.dma_start(qt_in[:], hs[ss * 128:(ss + 1) * 128, 0:512])
                sc_in = wpool.tile([128, 1], dt.float32, tag="sc_in")
                nc.sync.dma_start(sc_in[:], hsf[ss * 128:(ss + 1) * 128, 128:129])
                # integrity echoes: per-row sum of received payload bytes
                # (exact in f32: <= 512*238 < 2^24) and the received scale
                conv_in = wpool.tile([128, 512], dt.float32, tag="conv_in")
                nc.vector.tensor_scalar_mul(conv_in[:], qt_in[:], 1.0)
                insum = wpool.tile([128, 1], dt.float32, tag="insum")
                nc.vector.reduce_sum(insum[:], conv_in[:], axis=mybir.AxisListType.X)
                nc.sync.dma_start(yqf[ss * 128:(ss + 1) * 128, 98:99], insum[:])
                nc.sync.dma_start(yqf[ss * 128:(ss + 1) * 128, 99:100], sc_in[:])
                # v1 = rne((b - 7.5)/16) == floor(b/16) since frac in [-.47,.47]
                v1u = wpool.tile([128, 512], dt.uint8, tag="v1u")
                nc.vector.tensor_scalar(v1u[:], qt_in[:], 7.5, 1.0 / 16.0,
                                        op0=mybir.AluOpType.subtract,
                                        op1=mybir.AluOpType.mult)
                t16 = wpool.tile([128, 512], dt.float32, tag="t16")
                nc.vector.tensor_scalar_mul(t16[:], v1u[:], 16.0)
                v0f = wpool.tile([128, 512], dt.float32, tag="v0f")
                nc.vector.tensor_sub(v0f[:], qt_in[:], t16[:])
                hsb = wpool.tile([128, D_MODEL], dt.bfloat16, tag="hsb")
                nc.vector.tensor_scalar(hsb[:, 0:512], v0f[:], 7.0, sc_in[:],
                                        op0=mybir.AluOpType.subtract,
                                        op1=mybir.AluOpType.mult)
                nc.vector.tensor_scalar(hsb[:, 512:1024], v1u[:], 7.0, sc_in[:],
                                        op0=mybir.AluOpType.subtract,
                                        op1=mybir.AluOpType.mult)
                for t in range(8):
                    tp = wpool.tile([128, 128], dt.bfloat16, tag="tpt")
                    nc.sync.dma_start_transpose(
                        tp[:], hsb[:, t * 128:(t + 1) * 128])
                    nc.sync.dma_start(
                        hTs_d[t * 128:(t + 1) * 128, ss * 128:(ss + 1) * 128],
                        tp[:])
            nc.gpsimd.collective_compute(
                "AllGather", mybir.AluOpType.bypass,
                replica_groups=GRPS,
                ins=[hTs_d[:].opt()], outs=[hTg_d[:].opt()])

            # ---- load constants / weights; assemble hT (own batch) ----
            hT_sb = cpool.tile([128, 8 * SLEN], dt.bfloat16, tag="hT")
            for t in range(8):
                for i in range(4):
                    nc.sync.dma_start(
                        hT_sb[:, t * SLEN + i * ROWS: t * SLEN + (i + 1) * ROWS],
                        hTg_d[i * D_MODEL + t * 128: i * D_MODEL + (t + 1) * 128, :])
            Wq_sb = cpool.tile([128, 8 * 256], dt.bfloat16, tag="Wq")
            Wk_sb = cpool.tile([128, 8 * 256], dt.bfloat16, tag="Wk")
            Wvb_sb = cpool.tile([128, 8 * 260], dt.bfloat16, tag="Wvb")
            for t in range(8):
                nc.sync.dma_start(Wq_sb[:, t * 256:(t + 1) * 256], Wq[t * 128:(t + 1) * 128, :])
                nc.sync.dma_start(Wk_sb[:, t * 256:(t + 1) * 256], Wk[t * 128:(t + 1) * 128, :])
                nc.sync.dma_start(Wvb_sb[:, t * 260:(t + 1) * 260], Wvb[t * 128:(t + 1) * 128, :])
            pmA_sb = cpool.tile([128, P2M], dt.bfloat16, tag="pmA")
            nc.sync.dma_start(pmA_sb[:], pmA[:])
            maskS_sb = cpool.tile([128, 512], dt.float32, tag="maskS")
            maskI_sb = cpool.tile([128, 512], dt.float32, tag="maskI")
            nc.sync.dma_start(maskS_sb[:], maskS[:])
            nc.sync.dma_start(maskI_sb[:], maskI[:])
            WoB_sb = cpool.tile([128, 2 * D_MODEL], dt.bfloat16, tag="WoB")
            for t in range(2):
                nc.sync.dma_start(WoB_sb[:, t * D_MODEL:(t + 1) * D_MODEL],
                                  WoB[t * 128:(t + 1) * 128, :])

            # ---- phase A: xn_aug per head (128 rows = [xn(64); xn^2(64)]) ----
            xq = [fpool.tile([128, SLEN], dt.bfloat16, tag=f"xq{h}", name=f"xq{h}") for h in range(HPC)]
            xk = [fpool.tile([128, SLEN], dt.bfloat16, tag=f"xk{h}", name=f"xk{h}") for h in range(HPC)]
            for g in range(2):          # head group (2 heads)
                for lt in range(4):     # l tiles of 512
                    qps = psprj.tile([128, 512], dt.float32, tag="prj")
                    for kt in range(8):
                        nc.tensor.matmul(
                            qps[:],
                            lhsT=Wq_sb[:, kt * 256 + g * 128: kt * 256 + (g + 1) * 128],
                            rhs=hT_sb[:, kt * SLEN + lt * 512: kt * SLEN + (lt + 1) * 512],
                            start=(kt == 0), stop=(kt == 7))
                    for hh in range(2):
                        h = g * 2 + hh
                        sl = qps[hh * 64:(hh + 1) * 64, :]
                        nc.vector.tensor_scalar_mul(
                            xq[h][0:64, lt * 512:(lt + 1) * 512], sl, cxn)
                        nc.scalar.activation(
                            xq[h][64:128, lt * 512:(lt + 1) * 512], sl,
                            AF.Square, scale=cxn)
                    kps = psprj.tile([128, 512], dt.float32, tag="prj")
                    for kt in range(8):
                        nc.tensor.matmul(
                            kps[:],
                            lhsT=Wk_sb[:, kt * 256 + g * 128: kt * 256 + (g + 1) * 128],
                            rhs=hT_sb[:, kt * SLEN + lt * 512: kt * SLEN + (lt + 1) * 512],
                            start=(kt == 0), stop=(kt == 7))
                    for hh in range(2):
                        h = g * 2 + hh
                        sl = kps[hh * 64:(hh + 1) * 64, :]
                        nc.vector.tensor_scalar_mul(
                            xk[h][0:64, lt * 512:(lt + 1) * 512], sl, cxn)
                        nc.scalar.activation(
                            xk[h][64:128, lt * 512:(lt + 1) * 512], sl,
                            AF.Square, scale=cxn)

            # ---- scan state + transposed outputs ----
            st_ps = [psb.tile([128, 512], dt.float32, tag=f"st{i}", name=f"st{i}") for i in range(2)]
            st_sb = fpool.tile([128, 1024], dt.bfloat16, tag="st_sb")
            nc.vector.memset(st_sb[:], 0.0)
            oT_sb = [fpool.tile([128, SLEN], dt.bfloat16, tag=f"oT{t}", name=f"oT{t}")
                     for t in range(2)]

            for c in range(NCHUNK):
                first = (c == 0)
                # v/beta projection for this chunk: (128 l, 260)
                vps = psv.tile([128, 260], dt.float32, tag="vps")
                for kt in range(8):
                    nc.tensor.matmul(
                        vps[:],
                        lhsT=hT_sb[:, kt * SLEN + c * 128: kt * SLEN + (c + 1) * 128],
                        rhs=Wvb_sb[:, kt * 260:(kt + 1) * 260],
                        start=(kt == 0), stop=(kt == 7))
                beta = spool.tile([128, 4], dt.float32, tag="beta")
                nc.scalar.activation(beta[:], vps[:, 256:260], AF.Sigmoid)

                # features per head
                ktm, qtm, kqfm = [], [], []
                sigk = spool.tile([128, 4], dt.float32, tag="sigk")
                sigq = spool.tile([128, 4], dt.float32, tag="sigq")
                for h in range(HPC):
                    prj = psprj.tile([128, 512], dt.float32, tag="prj")
                    nc.tensor.matmul(prj[:], lhsT=xk[h][:, c * 128:(c + 1) * 128],
                                     rhs=pmA_sb[:], start=True, stop=True)
                    kt_t = kqpool.tile([128, 512], dt.bfloat16, tag="ktm")
                    nc.scalar.activation(kt_t[:], prj[:], AF.Exp,
                                         accum_out=sigk[:, h:h + 1])
                    ktm.append(kt_t)
                    prq = psprj.tile([128, 512], dt.float32, tag="prj")
                    nc.tensor.matmul(prq[:], lhsT=xq[h][:, c * 128:(c + 1) * 128],
                                     rhs=pmA_sb[:], start=True, stop=True)
                    qt_t = kqpool.tile([128, 512], dt.bfloat16, tag="qtm")
                    nc.scalar.activation(qt_t[:], prq[:], AF.Exp,
                                         accum_out=sigq[:, h:h + 1])
                    qtm.append(qt_t)
                    fm = kqpool.tile([128, 1024], dt.bfloat16, tag="kqfm")
                    for t in range(4):
                        nc.sync.dma_start_transpose(
                            fm[:, t * 128:(t + 1) * 128],
                            kt_t[:, t * 128:(t + 1) * 128])
                        nc.sync.dma_start_transpose(
                            fm[:, 512 + t * 128: 512 + (t + 1) * 128],
                            qt_t[:, t * 128:(t + 1) * 128])
                    kqfm.append(fm)

                # per-token scalars
                skp = spool.tile([128, 4], dt.float32, tag="skp")
                nc.vector.tensor_scalar_add(skp[:], sigk[:], P2M * PRIME_EPS)
                rk = spool.tile([128, 4], dt.float32, tag="rk")
                nc.vector.reciprocal(rk[:], skp[:])
                bp = spool.tile([128, 4], dt.float32, tag="bp")
                nc.vector.tensor_mul(bp[:], rk[:], rk[:])
                nc.vector.tensor_mul(bp[:], bp[:], beta[:])
                sqp = spool.tile([128, 4], dt.float32, tag="sqp")
                nc.vector.tensor_scalar_add(sqp[:], sigq[:], P2M * PRIME_EPS)
                rq = spool.tile([128, 4], dt.float32, tag="rq")
                nc.vector.reciprocal(rq[:], sqp[:])
                nc.vector.tensor_scalar_mul(rq[:], rq[:], float(D_HEAD ** -0.5))

                # G | GQ  (per head cols h*256: [G 128 | GQ 128])
                ggq = psb.tile([128, 1024], dt.float32, tag="ggq")
                for h in range(HPC):
                    for t in range(4):
                        rhs = kqfm[h][:].rearrange(
                            "p (two x) -> p two x", two=2)[:, :, t * 128:(t + 1) * 128]
                        nc.tensor.matmul(
                            ggq[:, h * 256:(h + 1) * 256],
                            lhsT=kqfm[h][:, t * 128:(t + 1) * 128],
                            rhs=rhs,
                            start=(t == 0 and h % 2 == 0), stop=(t == 3 and h % 2 == 1))
                # masked copies: Gm (strict upper), M2 (incl upper)
                gm = spool.tile([128, 512], dt.bfloat16, tag="gm")
                m2 = spool.tile([128, 512], dt.bfloat16, tag="m2")
                g_src = ggq[:].rearrange("p (h x) -> p h x", x=256)
                nc.vector.tensor_mul(
                    gm[:].rearrange("p (h x) -> p h x", x=128),
                    g_src[:, :, 0:128],
                    maskS_sb[:].rearrange("p (h x) -> p h x", x=128))
                nc.vector.tensor_mul(
                    m2[:].rearrange("p (h x) -> p h x", x=128),
                    g_src[:, :, 128:256],
                    maskI_sb[:].rearrange("p (h x) -> p h x", x=128))

                # KS | QS(+O)
                ksqs = psb.tile([128, 512], dt.float32, tag="ksqs")
                for h in range(HPC):
                    for t in range(4):
                        nc.tensor.matmul(
                            ksqs[:, h * 64:(h + 1) * 64],
                            lhsT=kqfm[h][:, t * 128:(t + 1) * 128],
                            rhs=st_sb[:, h * 256 + t * 64: h * 256 + (t + 1) * 64],
                            start=(h == 0 and t == 0), stop=False)
                for h in range(HPC):
                    for t in range(4):
                        nc.tensor.matmul(
                            ksqs[:, 256 + h * 64: 256 + (h + 1) * 64],
                            lhsT=kqfm[h][:, 512 + t * 128: 512 + (t + 1) * 128],
                            rhs=st_sb[:, h * 256 + t * 64: h * 256 + (t + 1) * 64],
                            start=False, stop=False)

                # B = bp * (skp * v - KS)   (per head, bf16)
                bmat = spool.tile([128, 256], dt.bfloat16, tag="bmat")
                tmp1 = spool.tile([128, 256], dt.float32, tag="tmp1")
                for h in range(HPC):
                    nc.vector.tensor_scalar_mul(
                        tmp1[:, h * 64:(h + 1) * 64],
                        vps[:, h * 64:(h + 1) * 64], skp[:, h:h + 1])
                for h in range(HPC):
                    nc.vector.tensor_sub(
                        tmp1[:, h * 64:(h + 1) * 64],
                        tmp1[:, h * 64:(h + 1) * 64],
                        ksqs[:, h * 64:(h + 1) * 64])
                for h in range(HPC):
                    nc.vector.tensor_scalar_mul(
                        bmat[:, h * 64:(h + 1) * 64],
                        tmp1[:, h * 64:(h + 1) * 64], bp[:, h:h + 1])

                # Neumann: X <- B - bp*(Gm^T.T @ X)
                x_cur = bmat
                for it in range(NEUMANN):
                    ax = psv.tile([128, 260], dt.float32, tag="vps", name="ax")
                    for h in range(HPC):
                        nc.tensor.matmul(
                            ax[:, h * 64:(h + 1) * 64],
                            lhsT=gm[:, h * 128:(h + 1) * 128],
                            rhs=x_cur[:, h * 64:(h + 1) * 64],
                            start=(h == 0), stop=(h == 3))
                    x_new = spool.tile([128, 256], dt.bfloat16, tag=f"x{it}")
                    for h in range(HPC):
                        nc.vector.tensor_scalar_mul(
                            tmp1[:, h * 64:(h + 1) * 64],
                            ax[:, h * 64:(h + 1) * 64], bp[:, h:h + 1])
                    nc.vector.tensor_sub(x_new[:], bmat[:], tmp1[:])
                    x_cur = x_new

                # O += tril(QK^T,0) @ U   (accumulate onto QS half of ksqs)
                for h in range(HPC):
                    nc.tensor.matmul(
                        ksqs[:, 256 + h * 64: 256 + (h + 1) * 64],
                        lhsT=m2[:, h * 128:(h + 1) * 128],
                        rhs=x_cur[:, h * 64:(h + 1) * 64],
                        start=False, stop=(h == 3))
                # out = O * rq  (bf16), then transpose into oT_sb
                o_sb = opool.tile([128, 256], dt.bfloat16, tag="o_sb")
                for h in range(HPC):
                    nc.vector.tensor_scalar_mul(
                        o_sb[:, h * 64:(h + 1) * 64],
                        ksqs[:, 256 + h * 64: 256 + (h + 1) * 64], rq[:, h:h + 1])
                for t in range(2):
                    nc.sync.dma_start_transpose(
                        oT_sb[t][:, c * 128:(c + 1) * 128],
                        o_sb[:, t * 128:(t + 1) * 128])

                # S update: st += K^T @ U ; refresh st_sb (bf16)
                for h in range(HPC):
                    for t in range(4):
                        nc.tensor.matmul(
                            st_ps[h // 2][:, (h % 2) * 256 + t * 64: (h % 2) * 256 + (t + 1) * 64],
                            lhsT=ktm[h][:, t * 128:(t + 1) * 128],
                            rhs=x_cur[:, h * 64:(h + 1) * 64],
                            start=(first and h % 2 == 0 and t == 0), stop=False)
                if c < NCHUNK - 1:
                    nc.vector.tensor_copy(st_sb[:, 0:512], st_ps[0][:])
                    nc.vector.tensor_copy(st_sb[:, 512:1024], st_ps[1][:])

            # ---- P = oT^T @ WoB  (per seq chunk, all 2048 rows) ----
            for c in range(NCHUNK):
                p_sb = opool.tile([128, D_MODEL], dt.float32, tag="p_sb")
                for nt in range(2):
                    pp = psprj.tile([128, 512], dt.float32, tag="prj")
                    for t in range(2):
                        nc.tensor.matmul(
                            pp[:],
                            lhsT=oT_sb[t][:, c * 128:(c + 1) * 128],
                            rhs=WoB_sb[:, t * D_MODEL + nt * 512: t * D_MODEL + (nt + 1) * 512],
                            start=(t == 0), stop=(t == 1))
                    nc.vector.tensor_copy(p_sb[:, nt * 512:(nt + 1) * 512], pp[:])
                nc.sync.dma_start(P_d[c * 128:(c + 1) * 128, :], p_sb[:])

            # ---- grouped ReduceScatter over the 4 cores of each batch ----
            nc.gpsimd.collective_compute(
                "ReduceScatter", mybir.AluOpType.add,
                replica_groups=GRPS,
                ins=[P_d[:].opt()], outs=[R_d[:].opt()])

            # ---- int3 quant + pack of own 512 attn rows ----
            # q = rne(a*3/rmax)+3 in [0,6]; q = 4*v2 + r with v2 = rne((q-1.5)/4)
            # in {0,1} (margin 0.125) and r = q - 4*v2 in [0,3].
            for i in range(4):
                x_sb = lnpool.tile([128, D_MODEL], dt.float32, tag="x_sb")
                nc.sync.dma_start(x_sb[:], R_d[i * 128:(i + 1) * 128, :])
                rmax = lnpool.tile([128, 1], dt.float32, tag="rmax")
                nc.vector.tensor_reduce(rmax[:], x_sb[:], axis=mybir.AxisListType.X,
                                        op=mybir.AluOpType.max,
                                        apply_absolute_value=True)
                nc.vector.tensor_scalar(rmax[:], rmax[:], 1e-20, None,
                                        op0=mybir.AluOpType.max)
                rs = lnpool.tile([128, 1], dt.float32, tag="rs")
                nc.vector.reciprocal(rs[:], rmax[:])
                nc.vector.tensor_scalar_mul(rs[:], rs[:], 3.0)
                qv = lnpool.tile([128, D_MODEL], dt.uint8, tag="qv")
                nc.vector.tensor_scalar(qv[:], x_sb[:], rs[:], 3.0,
                                        op0=mybir.AluOpType.mult,
                                        op1=mybir.AluOpType.add)
                v2u = lnpool.tile([128, D_MODEL], dt.uint8, tag="v2u")
                nc.vector.tensor_scalar(v2u[:], qv[:], 1.5, 0.25,
                                        op0=mybir.AluOpType.subtract,
                                        op1=mybir.AluOpType.mult)
                t4 = lnpool.tile([128, D_MODEL], dt.float32, tag="t4")
                nc.vector.tensor_scalar_mul(t4[:], v2u[:], 4.0)
                rro = lnpool.tile([128, D_MODEL], dt.float32, tag="rro")
                nc.vector.tensor_sub(rro[:], qv[:], t4[:])
                qt = lnpool.tile([128, 384], dt.uint8, tag="qt")
                accr = lnpool.tile([128, 256], dt.float32, tag="accr")
                nc.vector.tensor_copy(accr[:], rro[:, 0:256])
                tmr = lnpool.tile([128, 256], dt.float32, tag="tmr")
                for m in range(1, 4):
                    nc.vector.tensor_scalar_mul(
                        tmr[:], rro[:, m * 256:(m + 1) * 256], float(4 ** m))
                    if m < 3:
                        nc.vector.tensor_add(accr[:], accr[:], tmr[:])
                    else:
                        nc.vector.tensor_add(qt[:, 0:256], accr[:], tmr[:])
                accv = lnpool.tile([128, 128], dt.float32, tag="accv")
                nc.vector.tensor_copy(accv[:], v2u[:, 0:128])
                tmv = lnpool.tile([128, 128], dt.float32, tag="tmv")
                for k in range(1, 8):
                    nc.vector.tensor_scalar_mul(
                        tmv[:], v2u[:, k * 128:(k + 1) * 128], float(2 ** k))
                    if k < 7:
                        nc.vector.tensor_add(accv[:], accv[:], tmv[:])
                    else:
                        nc.vector.tensor_add(qt[:, 256:384], accv[:], tmv[:])
                conv_out = lnpool.tile([128, 384], dt.float32, tag="conv_out")
                nc.vector.tensor_scalar_mul(conv_out[:], qt[:], 1.0)
                outsum = lnpool.tile([128, 1], dt.float32, tag="outsum")
                nc.vector.reduce_sum(outsum[:], conv_out[:], axis=mybir.AxisListType.X)
                nc.sync.dma_start(yq[i * 128:(i + 1) * 128, 0:384], qt[:])
                nc.sync.dma_start(yqf[i * 128:(i + 1) * 128, 96:97], rmax[:])
                nc.sync.dma_start(yqf[i * 128:(i + 1) * 128, 97:98], rmax[:])
                nc.sync.dma_start(yqf[i * 128:(i + 1) * 128, 100:101], outsum[:])
    nc.compile()
    return nc


def _build_exec(nc):
    """Build a cached jitted SPMD executable around the bass program."""
    import jax
    import numpy as _np
    import concourse.mybir as mybir
    from concourse import bass2jax
    from jax.sharding import Mesh, PartitionSpec
    from jax.experimental.shard_map import shard_map

    bass2jax.install_neuronx_cc_hook()
    partition_name = (nc.partition_id_tensor.name
                      if nc.partition_id_tensor else None)
    in_names, out_names, out_shapes, out_dtypes = [], [], [], []
    for alloc in nc.m.functions[0].allocations:
        if not isinstance(alloc, mybir.MemoryLocationSet):
            continue
        name = alloc.memorylocations[0].name
        if alloc.kind == "ExternalInput":
            if name != partition_name:
                in_names.append(name)
        elif alloc.kind == "ExternalOutput":
            out_shapes.append(tuple(alloc.tensor_shape))
            out_dtypes.append(mybir.dt.np(alloc.dtype))
            out_names.append(name)
    out_avals = [jax.core.ShapedArray(s, d) for s, d in zip(out_shapes, out_dtypes)]
    all_names = list(in_names) + list(out_names)
    if partition_name is not None:
        all_names.append(partition_name)
    n_params, n_outs = len(in_names), len(out_names)

    def _body(*args):
        operands = list(args)
        if partition_name is not None:
            operands.append(bass2jax.partition_id_tensor())
        outs = bass2jax._bass_exec_p.bind(
            *operands,
            out_avals=tuple(out_avals),
            in_names=tuple(all_names),
            out_names=tuple(out_names),
            lowering_input_output_aliases=(),
            sim_require_finite=True,
            sim_require_nnan=True,
            nc=nc,
        )
        return tuple(outs)

    devices = jax.devices()[:N_CORES]
    mesh = Mesh(_np.asarray(devices), ("core",))
    fn = jax.jit(
        shard_map(_body, mesh=mesh,
                  in_specs=(PartitionSpec("core"),) * (n_params + n_outs),
                  out_specs=(PartitionSpec("core"),) * n_outs,
                  check_rep=False),
        keep_unused=True)
    return fn, in_names, out_names, out_shapes, out_dtypes


def _const_inputs(W_qkvb, W_o, ln_gamma, ln_beta, proj_matrix):
    """Per-core constant tensors, concatenated over cores (host side)."""
    bf16 = ml_dtypes.bfloat16
    Wr = np.asarray(W_qkvb, np.float32).reshape(D_MODEL, N_HEAD, 3 * D_HEAD + 1)
    pm = np.asarray(proj_matrix, np.float32)

    pmA = np.zeros((128, P2M), np.float32)
    pmA[0:64, 0:256] = pm
    pmA[0:64, 256:512] = -pm
    pmA[64:128, :] = -0.5
    maskS = np.tile(np.triu(np.ones((128, 128), np.float32), 1), (1, 4))
    maskI = np.tile(np.triu(np.ones((128, 128), np.float32), 0), (1, 4))
    Wo = np.asarray(W_o, np.float32)

    Wq_l, Wk_l, Wvb_l, WoB_l = [], [], [], []
    for c in range(N_CORES):
        hb0 = 4 * (c // 2)
        Wq_l.append(Wr[:, hb0:hb0 + 4, 0:64].reshape(D_MODEL, 256))
        Wk_l.append(Wr[:, hb0:hb0 + 4, 64:128].reshape(D_MODEL, 256))
        Wvb_l.append(np.concatenate([
            Wr[:, hb0:hb0 + 4, 128:192].reshape(D_MODEL, 256),
            Wr[:, hb0:hb0 + 4, 192],
        ], axis=1))
        WoB_l.append(Wo[hb0 * 64: hb0 * 64 + 256, :])
    return {
        "Wq": np.concatenate(Wq_l, axis=0).astype(bf16),
        "Wk": np.concatenate(Wk_l, axis=0).astype(bf16),
        "Wvb": np.ascontiguousarray(np.concatenate(Wvb_l, axis=0)).astype(bf16),
        "pmA": np.tile(pmA.astype(bf16), (N_CORES, 1)),
        "maskS": np.tile(maskS, (N_CORES, 1)),
        "maskI": np.tile(maskI, (N_CORES, 1)),
        "WoB": np.concatenate(WoB_l, axis=0).astype(bf16),
        "yq": np.zeros((N_CORES * ROWS, 404), np.uint8),
    }


class _IntegrityError(RuntimeError):
    pass


def _drop_caches():
    for k in ("consts", "argtmpl", "argtmpl_consts", "compiled",
              "compiled_tmpl", "wref", "whost"):
        _cache.pop(k, None)


def kernel(h, W_qkvb, W_o, ln_gamma, ln_beta, proj_matrix):
    """Retry wrapper. Two failure classes: (a) transport drops ("worker hung
    up" / UNAVAILABLE) which recover within ~a minute -> back off and rebuild
    device state; (b) silent transfer corruption caught by the checksum
    echoes -> retry immediately (rebuild device state from the second
    occurrence in case a cached constant upload was corrupted)."""
    import time
    integ, other = 0, 0
    while True:
        try:
            return _kernel_impl(h, W_qkvb, W_o, ln_gamma, ln_beta, proj_matrix)
        except _IntegrityError:
            integ += 1
            if integ >= 4:
                raise
            if integ >= 2:
                _drop_caches()
        except Exception:
            other += 1
            if other >= 3:
                raise
            time.sleep(30 if other == 1 else 75)
            _drop_caches()


def _kernel_impl(h, W_qkvb, W_o, ln_gamma, ln_beta, proj_matrix):
    import jax
    from jax.sharding import Mesh, PartitionSpec, NamedSharding

    bf16 = ml_dtypes.bfloat16
    h = np.asarray(h, np.float32)

    if "nc" not in _cache:
        _cache["nc"] = _build_fused()
        (_cache["fn"], _cache["in_names"], _cache["out_names"],
         _cache["out_shapes"], _cache["out_dtypes"]) = _build_exec(_cache["nc"])

    # device-cache the constant inputs. Fast path: same array objects as the
    # cached call (strong refs held, so ids can't be recycled). Slow path:
    # content check, rebuilding the device cache if the weights changed.
    wcur = (W_qkvb, W_o, ln_gamma, ln_beta, proj_matrix)
    wref = _cache.get("wref")
    same = wref is not None and all(a is b for a, b in zip(wref, wcur))
    if not same and wref is not None:
        same = all(np.array_equal(a, b) for a, b in zip(_cache["whost"], wcur))
    if not same:
        consts = _const_inputs(W_qkvb, W_o, ln_gamma, ln_beta, proj_matrix)
        devices = jax.devices()[:N_CORES]
        mesh = Mesh(np.asarray(devices), ("core",))
        shard = NamedSharding(mesh, PartitionSpec("core"))
        _cache["consts"] = {k: jax.device_put(v, shard) for k, v in consts.items()}
        _cache["whost"] = tuple(np.asarray(x).copy() for x in wcur)
    _cache["wref"] = wcur

    # per-call shard of h: core c gets rows [(c//2)*512, +512) of batch c%2
    if "pool" not in _cache:
        from concurrent.futures import ThreadPoolExecutor
        _cache["pool"] = ThreadPoolExecutor(8)
    pool = _cache["pool"]
    h4 = h.reshape(4, ROWS, 2, D_MODEL)
    hs_all = np.empty((N_CORES * ROWS, D_MODEL // 2 + 4), np.uint8)
    hs_f32 = hs_all.view(np.float32)  # (N_CORES*ROWS, 129); col 128 = scale
    insum_host = np.empty((N_CORES * ROWS,), np.float32)

    if _quant_nb is not None:
        _quant_nb(h4, hs_all, hs_f32, insum_host)
    else:
        def _quant_in(c):
            j, b = c // 2, c % 2
            blk = h4[j, :, b, :]
            sc = np.abs(blk).max(axis=-1, keepdims=True) * np.float32(1.0 / 7.0)
            np.maximum(sc, np.float32(1e-30), out=sc)
            hs_f32[c * ROWS:(c + 1) * ROWS, 128:129] = sc
            q = np.floor(blk / sc + np.float32(7.5)).astype(np.uint8)
            pk = q[:, 0:512] + 16 * q[:, 512:1024]
            hs_all[c * ROWS:(c + 1) * ROWS, 0:512] = pk
            insum_host[c * ROWS:(c + 1) * ROWS] = pk.sum(
                axis=1, dtype=np.int64).astype(np.float32)
        list(pool.map(_quant_in, range(N_CORES)))

    if "argtmpl" not in _cache or _cache.get("argtmpl_consts") is not _cache["consts"]:
        names = _cache["in_names"] + _cache["out_names"]
        _cache["argtmpl"] = [None if n == "hs" else _cache["consts"][n]
                             for n in names]
        _cache["hs_idx"] = names.index("hs")
        _cache["argtmpl_consts"] = _cache["consts"]
    args = list(_cache["argtmpl"])
    args[_cache["hs_idx"]] = hs_all
    if _cache.get("compiled_tmpl") is not _cache["argtmpl"]:
        try:
            _cache["compiled"] = _cache["fn"].lower(*args).compile()
        except Exception:
            _cache["compiled"] = _cache["fn"]  # fall back to the jit path
        _cache["compiled_tmpl"] = _cache["argtmpl"]
    outs = _cache["compiled"](*args)
    yq_all = np.asarray(outs[0])  # (8*512, 404) uint8: int3 attn + metadata
    yq_f32 = yq_all.view(np.float32)

    # integrity validation (catches silent transport corruption; retried by
    # the wrapper). All compared sums are integer-exact in f32.
    meta = yq_f32[:, 96:101]
    ok = bool(np.isfinite(meta).all())
    ok = ok and bool((yq_f32[:, 96] == yq_f32[:, 97]).all())
    ok = ok and bool((yq_f32[:, 98] == insum_host).all())
    ok = ok and bool((yq_f32[:, 99] == hs_f32[:, 128]).all())
    if ok:
        pk_sum = yq_all[:, 0:384].sum(axis=1, dtype=np.int64)
        ok = bool((pk_sum == yq_f32[:, 100].astype(np.int64)).all())
    if not ok:
        raise _IntegrityError("transfer checksum mismatch")

    gamma = np.ascontiguousarray(np.asarray(ln_gamma, np.float32))
    beta = np.ascontiguousarray(np.asarray(ln_beta, np.float32))
    out = np.empty((SLEN, BSZ, D_MODEL), np.float32)
    if _post_nb is not None:
        _post_nb(yq_all, yq_f32, h4, gamma, beta, out)
    else:
        sc_all = yq_f32[:, 96:97] * np.float32(1.0 / 3.0)

        def _post(c):
            j, b = c // 2, c % 2
            rpl = yq_all[c * ROWS:(c + 1) * ROWS, 0:256]
            vpl = yq_all[c * ROWS:(c + 1) * ROWS, 256:384]
            q = np.empty((ROWS, D_MODEL), np.int16)
            for k in range(8):
                m, lo = k >> 1, (k & 1) * 128
                rr = (rpl[:, lo:lo + 128] >> np.uint8(2 * m)) & np.uint8(3)
                v2 = (vpl >> np.uint8(k)) & np.uint8(1)
                q[:, k * 128:(k + 1) * 128] = 4 * v2.astype(np.int16) + rr
            a = (q.astype(np.float32) - np.float32(3.0)) * sc_all[c * ROWS:(c + 1) * ROWS]
            x = h4[j, :, b, :] + a
            mu = x.mean(-1, keepdims=True, dtype=np.float32)
            var = np.square(x - mu).mean(-1, keepdims=True, dtype=np.float32)
            y = (x - mu) / np.sqrt(var + np.float32(LN_EPS))
            y = y * gamma + beta
            out[j * ROWS:(j + 1) * ROWS, b, :] = y
        list(pool.map(_post, range(N_CORES)))
    return out

